# revision 32
# baseline (speedup 1.0000x reference)
"""Causal multi-head attention kernel for 8 trn2 NeuronCores.

Problem: x[2,2048,1024], 16 heads of dim 64, causal softmax(q k^T / sqrt(1024)) v,
then output projection. Sharding: data-parallel over batch (4 cores per batch),
tensor-parallel over heads (4 heads per core). Each core produces a partial
output (its heads' contribution through Wout); the host sums the 4 partials per
batch and adds b_out.

Per-core device program (SPMD, per-core data arrives via input tensors):
  1. DMA x[b] in, transpose on the PE -> xT [d on partitions, n free].
  2. Projections: qT/kT [dh on partitions, n free] (2-head groups of 128
     partitions), v natural [n on partitions] augmented with a ones column so
     the attention matmul also produces softmax row-sums.
  3. Per head, per 1024-wide i-chunk: S^T[j-block, i] = kT_j^T . qT_i on PE,
     exp((1/32) S) on ACT straight out of PSUM (logits are bounded, no
     max-subtraction needed), triangular mask multiply only on the diagonal
     128x128 block, then O^T[dh+1, i] += V_aug^T . P^T accumulated over
     j-blocks in PSUM. Block-causality skips all j>i blocks.
  4. Normalize by the row-sums (reciprocal + broadcast multiply), output
     projection with two 2-head pairs packed to a full K=128 contraction.
All matmuls run as float32r (relaxed fp32, full PE rate at N>=256).
"""

import os

import numpy as np

B, N, D, H = 2, 2048, 1024, 16
DH = D // H  # 64
SCALE = float(D) ** -0.5
NCORES = 8
HPC = 4  # heads per core
IC = 1024  # i-chunk width in attention phase
NB = N // 128  # 16 j/n blocks
KT = D // 128  # 8 contraction tiles
# v columns per head: 64 data cols + 64 ones cols. The ones columns make the
# attention matmul write the softmax row-sum replicated on PSUM partitions
# 64..127, so normalization is a plain elementwise reciprocal+multiply
# (partitions 0..63 / partitions 64..127) with no partition-broadcast needed.
VW = 2 * DH  # 128

_cached = {}
_last_results = None


def _build_program():
    import concourse.bacc as bacc
    import concourse.mybir as mybir
    import concourse.tile as tile
    from concourse.masks import make_identity

    f32 = mybir.dt.float32
    f32r = mybir.dt.float32r
    EXP = mybir.ActivationFunctionType.Exp
    LN = mybir.ActivationFunctionType.Ln

    nc = bacc.Bacc()

    xb = nc.dram_tensor("xb", [N, D], f32r, kind="ExternalInput")
    wq = nc.dram_tensor("wq", [D, HPC * DH], f32r, kind="ExternalInput")
    wk = nc.dram_tensor("wk", [D, HPC * DH], f32r, kind="ExternalInput")
    wv = nc.dram_tensor("wv", [D, HPC * DH], f32r, kind="ExternalInput")
    wo = nc.dram_tensor("wo", [HPC * DH, D], f32r, kind="ExternalInput")
    tri = nc.dram_tensor("tri", [128, 128], f32, kind="ExternalInput")
    outp = nc.dram_tensor("outp", [N, D], f32, kind="ExternalOutput")

    with tile.TileContext(nc) as tc:
        with (
            tc.tile_pool(name="const", bufs=1) as const_pool,
            tc.tile_pool(name="wts", bufs=1) as wts_pool,
            tc.tile_pool(name="big", bufs=1) as big_pool,
        ):
            ident = const_pool.tile([128, 128], f32, name="ident", tag="ident")
            make_identity(nc, ident)
            ident_r = const_pool.tile([128, 128], f32r, name="ident_r", tag="ident_r")
            nc.vector.tensor_copy(out=ident_r, in_=ident)
            # dummy exp: pulls the ACT exp-table load into the idle prologue
            # instead of stalling the first real exp mid-pipeline
            warm_exp = const_pool.tile([128, 1], f32, name="warm_exp", tag="warm_exp")
            nc.scalar.activation(out=warm_exp, in_=ident[:, 0:1], func=EXP)
            tri_sb = const_pool.tile([128, 128], f32, name="tri_sb", tag="tri_sb")
            nc.sync.dma_start(out=tri_sb, in_=tri[:, :])
            wo_sb = []
            for g in range(2):
                t = const_pool.tile([128, D], f32r, name=f"wo{g}", tag=f"wo{g}")
                nc.sync.dma_start(out=t, in_=wo[128 * g : 128 * (g + 1), :])
                wo_sb.append(t)

            wq_sb, wk_sb, wv_sb = [], [], []
            for nm, dram, lst in (("wq", wq, wq_sb), ("wk", wk, wk_sb), ("wv", wv, wv_sb)):
                for r in range(KT):
                    t = wts_pool.tile(
                        [128, HPC * DH], f32r, name=f"{nm}{r}", tag=f"{nm}{r}"
                    )
                    nc.sync.dma_start(out=t, in_=dram[128 * r : 128 * (r + 1), :])
                    lst.append(t)

            with (
                tc.tile_pool(name="xtp", bufs=1) as xt_pool,
                tc.tile_pool(name="pj", bufs=2, space="PSUM") as pj_pool,
                tc.tile_pool(name="osb", bufs=3) as osb_pool,
            ):
                # ---------------- phase 1: transpose x -> xT ----------------
                xT = []
                for r in range(KT):
                    t = xt_pool.tile([128, N], f32r, name=f"xT{r}", tag=f"xT{r}")
                    xT.append(t)
                with (
                    tc.tile_pool(name="stage", bufs=5) as stage_pool,
                    tc.tile_pool(name="pt", bufs=3, space="PSUM") as pt_pool,
                ):
                    for gq in range(NB // 4):
                        xs = []
                        for q in range(4):
                            t = stage_pool.tile([128, D], f32r, name="xs", tag="xs")
                            nb = 4 * gq + q
                            nc.sync.dma_start(
                                out=t, in_=xb[128 * nb : 128 * (nb + 1), :]
                            )
                            xs.append(t)
                        for r in range(KT):
                            ptr = pt_pool.tile([128, 512], f32, name="ptr", tag="ptr")
                            for q in range(4):
                                nc.tensor.transpose(
                                    ptr[:, 128 * q : 128 * (q + 1)].bitcast(f32r),
                                    xs[q][:, 128 * r : 128 * (r + 1)],
                                    ident_r,
                                )
                            nc.any.tensor_copy(
                                out=xT[r][:, 512 * gq : 512 * (gq + 1)], in_=ptr
                            )

                # ---------------- phase 2: projections ----------------
                qT, kT_ = [], []
                for g in range(2):
                    tq = big_pool.tile([128, N], f32r, name=f"qT{g}", tag=f"qT{g}")
                    tk = big_pool.tile([128, N], f32r, name=f"kT{g}", tag=f"kT{g}")
                    qT.append(tq)
                    kT_.append(tk)
                v_all = big_pool.tile(
                    [128, NB * HPC * VW], f32r, name="v_all", tag="v_all"
                )
                # ones columns for the row-sum trick: fill the whole tile with
                # 1.0; the projection copies below overwrite the data columns
                nc.vector.memset(v_all.bitcast(f32), 1.0)

                def vproj_stream():
                    for nb in range(NB):
                        pv = pj_pool.tile([128, HPC * DH], f32, name="pv", tag="pj")
                        for r in range(KT):
                            nc.tensor.matmul(
                                pv,
                                lhsT=xT[r][:, 128 * nb : 128 * (nb + 1)],
                                rhs=wv_sb[r],
                                start=(r == 0),
                                stop=(r == KT - 1),
                            )
                        base = nb * HPC * VW
                        for h in range(HPC):
                            nc.any.tensor_copy(
                                out=v_all[:, base + VW * h : base + VW * h + DH],
                                in_=pv[:, DH * h : DH * (h + 1)],
                            )
                        yield

                def qkproj_stream(g):
                    for s4 in range(4):
                        sl = slice(512 * s4, 512 * (s4 + 1))
                        for w_sb, dst in ((wq_sb, qT[g]), (wk_sb, kT_[g])):
                            pq = pj_pool.tile([128, 512], f32, name="pq", tag="pj")
                            for r in range(KT):
                                nc.tensor.matmul(
                                    pq,
                                    lhsT=w_sb[r][:, 128 * g : 128 * (g + 1)],
                                    rhs=xT[r][:, sl],
                                    start=(r == 0),
                                    stop=(r == KT - 1),
                                )
                            nc.any.tensor_copy(out=dst[:, sl], in_=pq)
                            yield

                for _ in vproj_stream():
                    pass
                qk0fill = qkproj_stream(0)
                for _ in range(4):
                    next(qk0fill)

                # ---------------- phase 3 + 4: attention with interleaved
                # ---------------- g1 projections and output projection ------
                OT = []
                for g in range(2):
                    t = big_pool.tile([128, N], f32r, name=f"OT{g}", tag=f"OT{g}")
                    OT.append(t)

                def outproj_stream(nbs):
                    for nb in nbs:
                        nsl = slice(128 * nb, 128 * (nb + 1))
                        for s in range(2):
                            po = pj_pool.tile([128, 512], f32, name="po", tag="pj")
                            for g in range(2):
                                nc.tensor.matmul(
                                    po,
                                    lhsT=OT[g][:, nsl],
                                    rhs=wo_sb[g][:, 512 * s : 512 * (s + 1)],
                                    start=(g == 0),
                                    stop=(g == 1),
                                )
                            ob = osb_pool.tile([128, 512], f32, name="ob", tag="osb")
                            nc.any.tensor_copy(out=ob, in_=po)
                            nc.sync.dma_start(
                                out=outp[nsl, 512 * s : 512 * (s + 1)], in_=ob
                            )
                            yield

                with (
                    tc.tile_pool(name="pS", bufs=2, space="PSUM") as pS_pool,
                    tc.tile_pool(name="pO", bufs=1, space="PSUM") as pO_pool,
                    tc.tile_pool(name="att", bufs=4) as att_pool,
                ):
                    # A@V emission lags the QK/exp emission by DELAY jb-steps
                    # so the in-order PE never stalls on the ACT exp; the
                    # PE-dense projection streams above are pulled in between
                    # attention steps to fill the remaining PE idle time.
                    DELAY = 2
                    pend = []

                    def drain(n):
                        while len(pend) > n:
                            pend.pop(0)()

                    def attention_stream():
                        for h in range(HPC):
                            g, row = h // 2, 64 * (h % 2)
                            for cp in range(2):
                                jd, jmax = 8 * cp, 8 * (cp + 1)
                                pO = pO_pool.tile(
                                    [128, IC], f32, name=f"pO{h}", tag="pO"
                                )
                                for jb in range(jmax):
                                    rel = jb - jd
                                    o = 128 * rel if rel > 0 else 0
                                    jsl = slice(128 * jb, 128 * (jb + 1))
                                    pS = pS_pool.tile(
                                        [128, IC], f32, name="pS", tag="pS"
                                    )
                                    pexp = att_pool.tile(
                                        [128, IC], f32r, name="pexp", tag="pexp"
                                    )
                                    # S^T = kT_j^T . qT_i (512-wide segments,
                                    # padded so N>=256 keeps f32r full rate;
                                    # padded cols are never read)
                                    for s in range(2):
                                        ls = o - 512 * s
                                        if ls >= 512:
                                            continue
                                        a = 512 * s + min(max(ls, 0), 256)
                                        nc.tensor.matmul(
                                            pS[:, a : 512 * (s + 1)],
                                            lhsT=kT_[g][row : row + 64, jsl],
                                            rhs=qT[g][
                                                row : row + 64,
                                                IC * cp + a : IC * cp + 512 * (s + 1),
                                            ],
                                            start=True,
                                            stop=True,
                                        )
                                    nc.scalar.activation(
                                        out=pexp[:, o:IC],
                                        in_=pS[:, o:IC],
                                        func=EXP,
                                        scale=SCALE,
                                    )
                                    if rel >= 0:
                                        nc.vector.tensor_mul(
                                            pexp[:, o : o + 128],
                                            pexp[:, o : o + 128],
                                            tri_sb,
                                        )

                                    def av_unit(
                                        h=h, jb=jb, o=o, jd=jd, jmax=jmax,
                                        pO=pO, pexp=pexp,
                                    ):
                                        # seg1 first (never overlaps the
                                        # masked triangle). Segments trimmed
                                        # to the causal offset, padded down
                                        # only to keep N>=256 (f32r full
                                        # rate); pad cols zeroed so they
                                        # accumulate 0.
                                        for s in (1, 0):
                                            hi = 512 * (s + 1)
                                            lo = max(o, 512 * s)
                                            if lo >= hi:
                                                continue
                                            lo = min(lo, hi - 256)
                                            if lo < o:
                                                nc.gpsimd.memset(
                                                    pexp[:, lo:o].bitcast(f32), 0.0
                                                )
                                            vsl = slice(
                                                jb * HPC * VW + VW * h,
                                                jb * HPC * VW + VW * (h + 1),
                                            )
                                            nc.tensor.matmul(
                                                pO[:, lo:hi],
                                                lhsT=v_all[:, vsl],
                                                rhs=pexp[:, lo:hi],
                                                start=(jb == 0),
                                                stop=(
                                                    jb
                                                    == (jd + 3 if s == 0 else jmax - 1)
                                                ),
                                                skip_group_check=True,
                                            )

                                    pend.append(av_unit)
                                    drain(DELAY)
                                    yield (h, cp, jb)

                                # Normalize: first a single fast copy of the
                                # whole O^T psum to SBUF (releases the pO slot
                                # for the next group after ~1us), then the
                                # reciprocal+multiply in 256-col chunks spread
                                # through the pend queue so no long DVE op
                                # blocks the tri-mask muls of the exp chain.
                                pOc = att_pool.tile(
                                    [128, IC], f32, name="pOc", tag="pOc", bufs=1
                                )

                                def copy_unit(pO=pO, pOc=pOc):
                                    nc.vector.tensor_copy(out=pOc, in_=pO)

                                pend.append(copy_unit)
                                for c4 in range(4):
                                    def norm_chunk(
                                        g=g, row=row, cp=cp, pOc=pOc, c4=c4
                                    ):
                                        cs = slice(256 * c4, 256 * (c4 + 1))
                                        rec = att_pool.tile(
                                            [64, 256], f32, name="rec", tag="rec",
                                            bufs=2,
                                        )
                                        nc.vector.reciprocal(
                                            out=rec, in_=pOc[64:128, cs]
                                        )
                                        nc.vector.tensor_mul(
                                            OT[g][
                                                row : row + 64,
                                                IC * cp + 256 * c4 : IC * cp
                                                + 256 * (c4 + 1),
                                            ],
                                            pOc[0:64, cs],
                                            rec,
                                        )

                                    pend.append(norm_chunk)

                    att = attention_stream()
                    g1fill = qkproj_stream(1)
                    out0fill = outproj_stream(range(8))
                    g1_done = False
                    steps = 0
                    for h, cp, jb in att:
                        steps += 1
                        if next(qk0fill, "end") != "end":
                            pass
                        elif not g1_done and steps % 3 == 0:
                            g1_done = next(g1fill, "end") == "end"
                        if h == 2 and not g1_done:
                            # g1 attention needs qT[1]/kT[1] complete
                            for _ in g1fill:
                                pass
                            g1_done = True
                        # nb 0..7 of the output projection read cp0 columns of
                        # OT; head 3's cp0 normalize leaves the pend queue at
                        # jb==DELAY of cp1, so only start pulling after that
                        if h == 3 and cp == 1 and jb > DELAY:
                            next(out0fill, None)
                            next(out0fill, None)
                    drain(0)
                    for _ in out0fill:
                        pass
                    for _ in outproj_stream(range(8, 16)):
                        pass

    nc.compile()
    return nc


def kernel(x, mask, Wq, Wkv, Wout, b_out):
    global _last_results
    from concourse.bass_utils import run_bass_kernel_spmd

    x = np.ascontiguousarray(np.asarray(x, dtype=np.float32))
    Wq = np.ascontiguousarray(np.asarray(Wq, dtype=np.float32))
    Wkv = np.ascontiguousarray(np.asarray(Wkv, dtype=np.float32))
    Wout = np.ascontiguousarray(np.asarray(Wout, dtype=np.float32))
    b_out = np.asarray(b_out, dtype=np.float32)

    if "nc" not in _cached:
        _cached["nc"] = _build_program()
    nc = _cached["nc"]

    jj, ii = np.mgrid[0:128, 0:128]
    tri = (jj <= ii).astype(np.float32)

    in_maps = []
    for c in range(NCORES):
        b = c // 4
        h0 = HPC * (c % 4)
        in_maps.append(
            {
                "xb": x[b],
                "wq": np.ascontiguousarray(Wq[:, DH * h0 : DH * (h0 + HPC)]),
                "wk": np.ascontiguousarray(Wkv[:, DH * h0 : DH * (h0 + HPC)]),
                "wv": np.ascontiguousarray(
                    Wkv[:, D + DH * h0 : D + DH * (h0 + HPC)]
                ),
                "wo": np.ascontiguousarray(Wout[DH * h0 : DH * (h0 + HPC), :]),
                "tri": tri,
            }
        )

    res = run_bass_kernel_spmd(
        nc,
        in_maps,
        core_ids=list(range(NCORES)),
        trace=bool(int(os.environ.get("KERNEL_TRACE", "0"))),
    )
    _last_results = res
    parts = [r["outp"] for r in res.results]
    out = np.empty((B, N, D), dtype=np.float32)
    for b in range(B):
        acc = parts[4 * b].astype(np.float32).copy()
        for c in range(4 * b + 1, 4 * b + 4):
            acc += parts[c]
        out[b] = acc + b_out[None, :]
    return out
